# revision 2
# baseline (speedup 1.0000x reference)
"""Trainium2 Bass kernel for nn_LossFunction_40346922778857 (v2).

Computes: scatter-loss over x (256,128,768).
  x1 = x[::2], x2 = x[1::2]  (each (128,128,768))
  per half: within (D,D), between (D,D) scatter matrices, corr-normalized,
  loss = sum((w1-w2)^2) + sum((b1-b2)^2).

Device computes per-half Gram G = X^T X (upper-triangle 128-row blocks) in
fp8e4 DoubleRow; per-b row sums S are computed on host from the SAME fp8
data (keeps within/between consistent with the quantized Gram).  Host sums
the 8 cores' partials and finishes the O(D^2) algebra in float64.

v2 layout (per core):
  in  x{h}: [128, 8(td), 2(s), 768] fp8, row (td*256+s*128+p) feature f
  out o{h}: [128, 2688] bf16, regions
      [i0a 512][i0b 256][i1a 512][i1b 128][i2 512][i3 384][i4 256][i5 128]
      region (i, colrange): G rows 128i:128(i+1), cols below.
"""

import numpy as np

P = 128
D = 768
NB = 16          # b's per half per core
TD = 8           # double-k-tiles per half (each contracts 256 rows)
NCORES = 8
OW = 2688        # output cols per half

_STATE = {}
LAST = {}

# (name, block i, col offset in G row-block, width, out col offset)
REGIONS = [
    ("i0a", 0, 0,   512, 0),
    ("i0b", 0, 512, 256, 512),
    ("i1a", 1, 128, 512, 768),
    ("i1b", 1, 640, 128, 1280),
    ("i2",  2, 256, 512, 1408),
    ("i3",  3, 384, 384, 1920),
    ("i4",  4, 512, 256, 2304),
    ("i5",  5, 640, 128, 2560),
]  # c0 = absolute G column start

# PSUM bank map per half: region -> (bank tile idx, col offset in bank)
BANKS = [
    # h0: warmup uses bank 6, h1 starts on the free banks 7/6
    {"i0a": (0, 0), "i0b": (1, 0), "i1a": (2, 0), "i1b": (1, 256),
     "i2": (3, 0), "i3": (4, 0), "i4": (5, 0), "i5": (5, 256)},
    {"i0a": (7, 0), "i0b": (6, 0), "i1a": (0, 0), "i1b": (6, 256),
     "i2": (2, 0), "i3": (3, 0), "i4": (4, 0), "i5": (4, 256)},
]
# within each td, emission order guarantees the bank-sharing start=True
# owner (i0b before i1b, i4 before i5) writes first.

# drain assignment: (region, engine) — DVE/ACT split, ~balanced cols
DRAIN_ENG = {"i0a": "v", "i0b": "a", "i1a": "a", "i1b": "a",
             "i2": "v", "i3": "a", "i4": "v", "i5": "v"}

# output DMAs per half: (out col start, out col end, regions covered)
OUT_CHUNKS = [
    (0, 768, ("i0a", "i0b")),
    (768, 1408, ("i1a", "i1b")),
    (1408, 2304, ("i2", "i3")),
    (2304, 2688, ("i4", "i5")),
]


def _build():
    import concourse.tile as tile
    from concourse import bacc, mybir

    nc = bacc.Bacc("TRN2", target_bir_lowering=False, debug=False,
                   num_devices=NCORES)

    fp8 = mybir.dt.float8e4
    xins = [nc.dram_tensor(f"x{h}", [P, TD, 2, D], fp8,
                           kind="ExternalInput").ap() for h in range(2)]
    outs = [nc.dram_tensor(f"o{h}", [P, OW], mybir.dt.bfloat16,
                           kind="ExternalOutput").ap() for h in range(2)]

    with tile.TileContext(nc) as tc:
        with tc.tile_pool(name="xp", bufs=2) as xp, \
             tc.tile_pool(name="wp", bufs=1) as wp, \
             tc.tile_pool(name="pp", bufs=8, space="PSUM") as pp, \
             tc.tile_pool(name="op", bufs=2) as op:
            xts = [xp.tile([P, TD, 2, D], fp8, tag="xt", name=f"xt{h}")
                   for h in range(2)]
            ots = [op.tile([P, OW], mybir.dt.bfloat16, tag="ot",
                           name=f"ot{h}") for h in range(2)]
            banks = [pp.tile([P, 512], mybir.dt.float32, tag="ps",
                             name=f"bank{b}") for b in range(8)]
            wt = wp.tile([P, 640], mybir.dt.float16, tag="wt")

            # input DMAs: 4 chunks of 2 td per half, alternating the two
            # HWDGE rings (sync + scalar) so both stream in parallel.
            for h in range(2):
                for c in range(4):
                    eng = nc.sync if (h * 4 + c) % 2 == 0 else nc.scalar
                    eng.dma_start(out=xts[h][:, 2 * c:2 * c + 2, :, :],
                                  in_=xins[h][:, 2 * c:2 * c + 2, :, :])

            # PE warm-up on zeros while inputs stream: ~3.4us of activity
            # brings the HAM clock gate to 8/8 as real matmuls start.
            nc.vector.memset(wt[:], 0.0)
            for _ in range(8):
                nc.tensor.matmul(banks[6][:, :512], wt[:, :128],
                                 wt[:, 128:640], start=True, stop=True)

            for h in range(2):
                xt = xts[h]
                bmap = BANKS[h]
                for td in range(TD):
                    for (name, i, c0, w, _oc) in REGIONS:
                        b, boff = bmap[name]
                        # bank-sharing: only the first writer of a shared
                        # bank uses start=True (start clears the whole
                        # 2KB bank, incl. the co-located region).
                        st = (td == 0) and boff == 0
                        lhsT = xt[:, td, :, 128 * i:128 * i + 128]
                        rhs = xt[:, td, :, c0:c0 + w]
                        nc.tensor.matmul(banks[b][:, boff:boff + w],
                                         lhsT, rhs,
                                         start=st, stop=(td == TD - 1),
                                         skip_group_check=True,
                                         perf_mode=mybir.MatmulPerfMode.DoubleRow)
                # drains: PSUM -> SBUF bf16 split across DVE + ACT
                ot = ots[h]
                for (name, i, c0, w, oc) in REGIONS:
                    b, boff = bmap[name]
                    src = banks[b][:, boff:boff + w]
                    dst = ot[:, oc:oc + w]
                    if DRAIN_ENG[name] == "v":
                        nc.vector.tensor_copy(dst, src)
                    else:
                        nc.scalar.copy(dst, src)
                # outputs stream per chunk on the sync ring
                for (a, bnd, _regs) in OUT_CHUNKS:
                    nc.sync.dma_start(out=outs[h][:, a:bnd],
                                      in_=ot[:, a:bnd])
    nc.compile()
    return nc


def _get_nc():
    if "nc" not in _STATE:
        _STATE["nc"] = _build()
    return _STATE["nc"]


def _quantize(x):
    import ml_dtypes
    return x.astype(np.float16).astype(ml_dtypes.float8_e4m3)


def _prep_half(x8h):
    """x8h: (128, 128, 768) fp8 for one half -> per-core [128, 8, 2, 768]."""
    out = []
    for c in range(NCORES):
        blk = x8h[NB * c:NB * (c + 1)]                    # (16, 128, 768)
        # b = 2*td + s, n = p  ->  (p, td, s, f)
        out.append(np.ascontiguousarray(
            blk.reshape(TD, 2, P, D).transpose(2, 0, 1, 3)))
    return out


def kernel(x, label=None, genre_label=None, _trace=False):
    from concourse.bass_utils import run_bass_kernel_spmd

    nc = _get_nc()

    x = np.asarray(x, dtype=np.float32)
    x8 = [_quantize(x[0::2]), _quantize(x[1::2])]
    halves = [_prep_half(x8[0]), _prep_half(x8[1])]
    in_maps = [{"x0": halves[0][c], "x1": halves[1][c]} for c in range(NCORES)]

    # First execution of a freshly compiled NEFF has been observed to be
    # flaky (garbage output or device error); validate and retry.
    res = None
    for attempt in range(3):
        try:
            res = run_bass_kernel_spmd(nc, in_maps, list(range(NCORES)),
                                       trace=_trace)
        except Exception:
            if attempt == 2:
                raise
            continue
        ok = all(
            np.isfinite(np.asarray(res.results[c][f"o{h}"],
                                   dtype=np.float32)).all()
            and np.any(np.asarray(res.results[c][f"o{h}"], dtype=np.float32))
            for c in range(NCORES) for h in range(2))
        if ok:
            break
    LAST["res"] = res

    B = x.shape[0] // 2
    N = x.shape[1]
    tol = B * N

    loss = 0.0
    for h in range(2):
        U = np.zeros((D, D), dtype=np.float64)
        for c in range(NCORES):
            o = np.asarray(res.results[c][f"o{h}"], dtype=np.float64)
            for (name, i, c0, w, oc) in REGIONS:
                U[128 * i:128 * (i + 1), c0:c0 + w] += o[:, oc:oc + w]
        G = np.zeros((D, D), dtype=np.float64)
        for i in range(6):
            ri = slice(P * i, P * (i + 1))
            G[ri, P * i:D] = U[ri, P * i:D]
            for j in range(i + 1, 6):
                rj = slice(P * j, P * (j + 1))
                G[rj, ri] = U[ri, rj].T
        # row sums from the same quantized data (consistent with G)
        S = x8[h].astype(np.float64).sum(axis=1)          # (B, D)
        xbar = S / N
        M = xbar.T @ xbar
        mean = xbar.mean(axis=0)
        within = (G - N * M) / tol
        between = N * (M - B * np.outer(mean, mean)) / tol
        w_h = within / np.sqrt(np.sum(np.diagonal(within) ** 2))
        b_h = between / np.sqrt(np.sum(np.diagonal(between) ** 2))
        if h == 0:
            w0, b0 = w_h, b_h
        else:
            loss = np.sum((w0 - w_h) ** 2) + np.sum((b0 - b_h) ** 2)
    return np.asarray(loss, dtype=np.float32)


# revision 3
# speedup vs baseline: 1.0054x; 1.0054x over previous
"""Trainium2 Bass kernel for nn_LossFunction_40346922778857 (v2).

Computes: scatter-loss over x (256,128,768).
  x1 = x[::2], x2 = x[1::2]  (each (128,128,768))
  per half: within (D,D), between (D,D) scatter matrices, corr-normalized,
  loss = sum((w1-w2)^2) + sum((b1-b2)^2).

Device computes per-half Gram G = X^T X (upper-triangle 128-row blocks) in
fp8e4 DoubleRow; per-b row sums S are computed on host from the SAME fp8
data (keeps within/between consistent with the quantized Gram).  Host sums
the 8 cores' partials and finishes the O(D^2) algebra in float64.

v2 layout (per core):
  in  x{h}: [128, 8(td), 2(s), 768] fp8, row (td*256+s*128+p) feature f
  out o{h}: [128, 2688] bf16, regions
      [i0a 512][i0b 256][i1a 512][i1b 128][i2 512][i3 384][i4 256][i5 128]
      region (i, colrange): G rows 128i:128(i+1), cols below.
"""

import numpy as np

P = 128
D = 768
NB = 16          # b's per half per core
TD = 8           # double-k-tiles per half (each contracts 256 rows)
NCORES = 8
OW = 2688        # output cols per half

_STATE = {}
LAST = {}

# (name, block i, col offset in G row-block, width, out col offset)
REGIONS = [
    ("i0a", 0, 0,   512, 0),
    ("i0b", 0, 512, 256, 512),
    ("i1a", 1, 128, 512, 768),
    ("i1b", 1, 640, 128, 1280),
    ("i2",  2, 256, 512, 1408),
    ("i3",  3, 384, 384, 1920),
    ("i4",  4, 512, 256, 2304),
    ("i5",  5, 640, 128, 2560),
]  # c0 = absolute G column start

# PSUM bank map per half: region -> (bank tile idx, col offset in bank)
# h0 packs into 6 banks (b1 and b5 shared) so h1 starts with b6/b7 free;
# within each td, emission order guarantees the bank-sharing start=True
# owner (i0b before i1b, i4 before i5) writes first.
# h1 uses 8 distinct banks, each fully ordered after its h0 readers via
# AP overlap (512-wide regions land on h0's shared banks so the whole-bank
# clear of start=True is covered by WAR deps on both h0 casts).
BANKS = [
    {"i0a": (0, 0), "i0b": (1, 0), "i1a": (2, 0), "i1b": (1, 256),
     "i2": (3, 0), "i3": (4, 0), "i4": (5, 0), "i5": (5, 256)},
    {"i0a": (7, 0), "i0b": (6, 0), "i1a": (1, 0), "i1b": (0, 0),
     "i2": (5, 0), "i3": (2, 0), "i4": (3, 0), "i5": (4, 0)},
]

# drain emission order + engine (v=DVE, a=ACT): ordered so the banks h1
# needs earliest are freed first on each engine.
DRAINS = [("i0a", "v"), ("i0b", "a"), ("i1b", "a"), ("i1a", "a"),
          ("i2", "v"), ("i3", "a"), ("i4", "v"), ("i5", "v")]

# output DMAs per half: (out col start, out col end, regions covered)
OUT_CHUNKS = [
    (0, 768, ("i0a", "i0b")),
    (768, 1408, ("i1a", "i1b")),
    (1408, 2304, ("i2", "i3")),
    (2304, 2688, ("i4", "i5")),
]


def _build():
    import concourse.tile as tile
    from concourse import bacc, mybir

    nc = bacc.Bacc("TRN2", target_bir_lowering=False, debug=False,
                   num_devices=NCORES)

    fp8 = mybir.dt.float8e4
    xins = [nc.dram_tensor(f"x{h}", [P, TD, 2, D], fp8,
                           kind="ExternalInput").ap() for h in range(2)]
    outs = [nc.dram_tensor(f"o{h}", [P, OW], mybir.dt.bfloat16,
                           kind="ExternalOutput").ap() for h in range(2)]

    with tile.TileContext(nc) as tc:
        with tc.tile_pool(name="xp", bufs=2) as xp, \
             tc.tile_pool(name="wp", bufs=1) as wp, \
             tc.tile_pool(name="pp", bufs=8, space="PSUM") as pp, \
             tc.tile_pool(name="op", bufs=2) as op:
            xts = [xp.tile([P, TD, 2, D], fp8, tag="xt", name=f"xt{h}")
                   for h in range(2)]
            ots = [op.tile([P, OW], mybir.dt.bfloat16, tag="ot",
                           name=f"ot{h}") for h in range(2)]
            banks = [pp.tile([P, 512], mybir.dt.float32, tag="ps",
                             name=f"bank{b}") for b in range(8)]
            wt = wp.tile([P, 640], mybir.dt.float16, tag="wt")

            # input DMAs: 4 chunks of 2 td per half, split so both HWDGE
            # rings (sync + scalar) carry half of each half's data and
            # deliver roughly in consumption order.
            for h in range(2):
                for eng, cc in ((nc.sync, 0), (nc.scalar, 2),
                                (nc.sync, 1), (nc.scalar, 3)):
                    eng.dma_start(out=xts[h][:, 2 * cc:2 * cc + 2, :, :],
                                  in_=xins[h][:, 2 * cc:2 * cc + 2, :, :])

            # PE warm-up on zeros while inputs stream: ~3.4us of activity
            # brings the HAM clock gate to 8/8 as real matmuls start.
            # memset on gpsimd: its stream is free at t=0.
            nc.gpsimd.memset(wt[:], 0.0)
            for _ in range(8):
                nc.tensor.matmul(banks[6][:, :512], wt[:, :128],
                                 wt[:, 128:640], start=True, stop=True)

            rdict = {r[0]: r for r in REGIONS}
            for h in range(2):
                xt = xts[h]
                bmap = BANKS[h]
                for td in range(TD):
                    for (name, i, c0, w, _oc) in REGIONS:
                        b, boff = bmap[name]
                        # bank-sharing (h0 only): the first writer of a
                        # shared bank uses start=True (start clears the
                        # whole 2KB bank incl. the co-located region).
                        st = (td == 0) and boff == 0
                        lhsT = xt[:, td, :, 128 * i:128 * i + 128]
                        rhs = xt[:, td, :, c0:c0 + w]
                        nc.tensor.matmul(banks[b][:, boff:boff + w],
                                         lhsT, rhs,
                                         start=st, stop=(td == TD - 1),
                                         skip_group_check=True,
                                         perf_mode=mybir.MatmulPerfMode.DoubleRow)
                # drains: PSUM -> SBUF bf16 split across DVE + ACT
                ot = ots[h]
                for name, e in DRAINS:
                    (_n, i, c0, w, oc) = rdict[name]
                    b, boff = bmap[name]
                    src = banks[b][:, boff:boff + w]
                    dst = ot[:, oc:oc + w]
                    if e == "v":
                        nc.vector.tensor_copy(dst, src)
                    else:
                        nc.scalar.copy(dst, src)
                # outputs stream per chunk; h0 on the scalar ring, h1 on
                # sync so h1's tail DMAs never queue behind h0's.
                oeng = nc.scalar if h == 0 else nc.sync
                for (a, bnd, _regs) in OUT_CHUNKS:
                    oeng.dma_start(out=outs[h][:, a:bnd],
                                   in_=ot[:, a:bnd])
    nc.compile()
    return nc


def _get_nc():
    if "nc" not in _STATE:
        _STATE["nc"] = _build()
    return _STATE["nc"]


def _quantize(x):
    import ml_dtypes
    return x.astype(np.float16).astype(ml_dtypes.float8_e4m3)


def _prep_half(x8h):
    """x8h: (128, 128, 768) fp8 for one half -> per-core [128, 8, 2, 768]."""
    out = []
    for c in range(NCORES):
        blk = x8h[NB * c:NB * (c + 1)]                    # (16, 128, 768)
        # b = 2*td + s, n = p  ->  (p, td, s, f)
        out.append(np.ascontiguousarray(
            blk.reshape(TD, 2, P, D).transpose(2, 0, 1, 3)))
    return out


def kernel(x, label=None, genre_label=None, _trace=False):
    from concourse.bass_utils import run_bass_kernel_spmd

    nc = _get_nc()

    x = np.asarray(x, dtype=np.float32)
    x8 = [_quantize(x[0::2]), _quantize(x[1::2])]
    halves = [_prep_half(x8[0]), _prep_half(x8[1])]
    in_maps = [{"x0": halves[0][c], "x1": halves[1][c]} for c in range(NCORES)]

    # First execution of a freshly compiled NEFF has been observed to be
    # flaky (garbage output or device error); validate and retry.
    res = None
    for attempt in range(3):
        try:
            res = run_bass_kernel_spmd(nc, in_maps, list(range(NCORES)),
                                       trace=_trace)
        except Exception:
            if attempt == 2:
                raise
            continue
        ok = all(
            np.isfinite(np.asarray(res.results[c][f"o{h}"],
                                   dtype=np.float32)).all()
            and np.any(np.asarray(res.results[c][f"o{h}"], dtype=np.float32))
            for c in range(NCORES) for h in range(2))
        if ok:
            break
    LAST["res"] = res

    B = x.shape[0] // 2
    N = x.shape[1]
    tol = B * N

    loss = 0.0
    for h in range(2):
        U = np.zeros((D, D), dtype=np.float64)
        for c in range(NCORES):
            o = np.asarray(res.results[c][f"o{h}"], dtype=np.float64)
            for (name, i, c0, w, oc) in REGIONS:
                U[128 * i:128 * (i + 1), c0:c0 + w] += o[:, oc:oc + w]
        G = np.zeros((D, D), dtype=np.float64)
        for i in range(6):
            ri = slice(P * i, P * (i + 1))
            G[ri, P * i:D] = U[ri, P * i:D]
            for j in range(i + 1, 6):
                rj = slice(P * j, P * (j + 1))
                G[rj, ri] = U[ri, rj].T
        # row sums from the same quantized data (consistent with G)
        S = x8[h].astype(np.float64).sum(axis=1)          # (B, D)
        xbar = S / N
        M = xbar.T @ xbar
        mean = xbar.mean(axis=0)
        within = (G - N * M) / tol
        between = N * (M - B * np.outer(mean, mean)) / tol
        w_h = within / np.sqrt(np.sum(np.diagonal(within) ** 2))
        b_h = between / np.sqrt(np.sum(np.diagonal(between) ** 2))
        if h == 0:
            w0, b0 = w_h, b_h
        else:
            loss = np.sum((w0 - w_h) ** 2) + np.sum((b0 - b_h) ** 2)
    return np.asarray(loss, dtype=np.float32)
